# revision 3
# baseline (speedup 1.0000x reference)
"""Trainium2 Bass kernel for nn_ConvEmbedding.

Computes out = L2normalize_rows(x @ W_band^T + b) where W_band is the
(E, D) banded scatter of the Conv1d weight w (E, K): W_band[i, i+k] = w[i, k].

Strategy (8 NeuronCores, data-parallel over batch N):
  - host: build WbT = W_band.T (D, E) once; shard x row-wise into 8 shards of
    (NSH, D); transpose each shard to xt (D, NSH) so the contraction dim d
    lands on SBUF partitions with no on-chip transpose.
  - device (per core): for each 128-row output tile, one DMA brings in the
    (D x 128) slab of xt; the bias is accumulated into PSUM via a K=1 matmul
    (ones^T @ b) that also initializes the accumulation group; then banded
    matmuls accumulate xt_tile^T @ WbT_tile, skipping all-zero band tiles.
    ScalarE does square+row-sum in one fused op, VectorE does
    max(sqrt, eps) reciprocal + scale, DMA writes the tile out.
"""

import os

import numpy as np

import concourse.mybir as mybir
import concourse.tile as tile
from concourse import bacc
from concourse.bass import ts
from concourse.bass_utils import run_bass_kernel_spmd

N, D, E, KW = 16384, 2048, 512, 1537
EPS = 1e-12
NCORES = 8
NSH = N // NCORES        # 2048 batch rows per core
NT = NSH // 128          # 16 output row tiles per core
KT = D // 128            # 16 contraction tiles

# float32r: single-pass fp32 matmul mode (full PE rate at free dim >= 256).
# float32: exact fp32 (2 half-speed passes -> 4x slower). Selectable for A/B.
_DT_BY_NAME = {"float32r": mybir.dt.float32r, "float32": mybir.dt.float32}
MM_DT = _DT_BY_NAME[os.environ.get("CONV_EMB_MM_DT", "float32r")]


def _band(kt: int) -> tuple[int, int]:
    """Nonzero e-column range [lo, hi) of WbT rows [128*kt, 128*kt+128)."""
    lo = max(0, 128 * kt - (KW - 1))
    hi = min(E, 128 * kt + 128)
    return lo, hi


def _band_widened(kt: int) -> tuple[int, int]:
    """Band range widened to >=256 cols (fp32r runs 4x slower below 256)."""
    lo, hi = _band(kt)
    if hi - lo < 256:
        if lo == 0:
            hi = min(E, 256)
        else:
            lo = max(0, hi - 256)
    return lo, hi


def build_nc(reps: int = 1, mm_dt=None):
    """Build the per-core Bass program (same SPMD program for all cores)."""
    if mm_dt is None:
        mm_dt = MM_DT
    nc = bacc.Bacc(None, target_bir_lowering=False)
    xt = nc.dram_tensor("xt", [D, NSH], mm_dt, kind="ExternalInput")
    wbt = nc.dram_tensor("wbt", [D, E], mm_dt, kind="ExternalInput")
    bias = nc.dram_tensor("bias", [1, E], mm_dt, kind="ExternalInput")
    out = nc.dram_tensor("out", [NSH, E], mybir.dt.float32, kind="ExternalOutput")

    xt_r = xt.rearrange("(kt p) n -> p kt n", p=128)
    wbt_r = wbt.rearrange("(kt p) e -> p kt e", p=128)

    with tile.TileContext(nc) as tc:
        with (
            tc.tile_pool(name="const", bufs=1) as cpool,
            tc.tile_pool(name="xin", bufs=3) as xpool,
            tc.tile_pool(name="res", bufs=3) as rpool,
            tc.tile_pool(name="psum", bufs=4, space="PSUM") as ppool,
        ):
            wbt_sb = cpool.tile([128, KT, E], mm_dt)
            nc.sync.dma_start(wbt_sb[:], wbt_r[:])
            bias_sb = cpool.tile([1, E], mm_dt)
            nc.sync.dma_start(bias_sb[:], bias[:])
            ones_f32 = cpool.tile([1, 128], mybir.dt.float32)
            nc.vector.memset(ones_f32[:], 1.0)
            ones_sb = ones_f32[:].bitcast(mm_dt) if mm_dt != mybir.dt.float32 else ones_f32[:]

            for _rep in range(reps):
                for i in range(NT):
                    xt_sb = xpool.tile([128, KT, 128], mm_dt, tag="xt")
                    nc.sync.dma_start(xt_sb[:], xt_r[:, :, ts(i, 128)])

                    ps = ppool.tile([128, E], mybir.dt.float32, tag="ps")
                    # bias row broadcast: ones(1,128)^T @ b(1,E); start=True
                    # clears the whole bank so banded tiles can accumulate.
                    nc.tensor.matmul(
                        ps[:], ones_sb, bias_sb[:],
                        start=True, stop=False, skip_group_check=True,
                    )
                    for kt in range(KT):
                        lo, hi = _band_widened(kt)
                        nc.tensor.matmul(
                            ps[:, lo:hi],
                            xt_sb[:, kt, :],
                            wbt_sb[:, kt, lo:hi],
                            start=False, stop=(kt == KT - 1),
                            skip_group_check=True,
                        )

                    sq = rpool.tile([128, E], mybir.dt.float32, tag="sq")
                    ss = rpool.tile([128, 1], mybir.dt.float32, tag="ss")
                    nc.scalar.activation(
                        sq[:], ps[:], mybir.ActivationFunctionType.Square,
                        accum_out=ss[:],
                    )
                    nrm = rpool.tile([128, 1], mybir.dt.float32, tag="nrm")
                    nc.scalar.sqrt(nrm[:], ss[:])
                    nc.vector.tensor_scalar_max(nrm[:], nrm[:], EPS)
                    inv = rpool.tile([128, 1], mybir.dt.float32, tag="inv")
                    nc.vector.reciprocal(inv[:], nrm[:])
                    ob = rpool.tile([128, E], mybir.dt.float32, tag="ob")
                    nc.vector.tensor_scalar_mul(ob[:], ps[:], inv[:])
                    nc.sync.dma_start(out[ts(i, 128), :], ob[:])
    nc.finalize()
    return nc


def build_wbt(w: np.ndarray) -> np.ndarray:
    """Scatter w (E, KW) into the transposed banded matrix WbT (D, E)."""
    wbt = np.zeros((D, E), np.float32)
    e_idx = np.arange(E)
    rows = (e_idx[:, None] + np.arange(KW)[None, :]).ravel()
    cols = np.repeat(e_idx, KW)
    wbt[rows, cols] = np.ascontiguousarray(w, dtype=np.float32).ravel()
    return wbt


def make_in_maps(x: np.ndarray, w: np.ndarray, b: np.ndarray):
    wbt = build_wbt(w)
    bias = np.ascontiguousarray(b, dtype=np.float32).reshape(1, E)
    xr = np.asarray(x, dtype=np.float32).reshape(NCORES, NSH, D)
    return [
        {"xt": np.ascontiguousarray(xr[c].T), "wbt": wbt, "bias": bias}
        for c in range(NCORES)
    ]


def kernel(x: np.ndarray, w: np.ndarray, b: np.ndarray) -> np.ndarray:
    in_maps = make_in_maps(x, w, b)
    nc = build_nc()
    res = run_bass_kernel_spmd(nc, in_maps, core_ids=list(range(NCORES)))
    return np.concatenate(
        [res.results[c]["out"] for c in range(NCORES)], axis=0
    )
